# revision 36
# baseline (speedup 1.0000x reference)
"""NNCLR allswap loss kernel for 8 Trainium2 NeuronCores.

Math (from the reference):
  p = l2norm(projected)  [B=2048, Vg=2, D=256]
  q = l2norm(predicted)  [B=2048, Vt=4, D=256]
  logits[i,j] = p[:,i] @ q[:,j].T / T           (T = 0.2)
  L[i,j] = mean_b( logsumexp_c(logits[i,j,b,:]) - logits[i,j,b,b] )
  Only L[:, :2] is used (Vl = 2), so predicted views 2,3 never touch
  the device.

Sharding: 8 cores = 4 (i,j) view pairs x 2 batch-row halves.  Core
(pair, h) owns rows [h*1024, (h+1)*1024) and ALL 2048 columns of its
logits matrix, so each row's sum_c exp(logit) completes on one core
(no cross-core combine for the logsumexp).  Columns are rotated by
h*1024 so the diagonal block of row-tile m sits at local columns
[m*128, (m+1)*128) on every core -> one SPMD program.

Device work per core (everything heavy):
  * 32 fp8(e4m3) DoubleRow matmuls: K=256 contracted in one pass
    (128 partitions x 2 k-tiles in the free dims of both operands).
  * 8 Exp activations over [128, 2048] PSUM with per-row scale
    5/(16*|p_row|) and fused row-sum accumulation (the ACT engine is
    the critical chain: ~2.2us per tile).
  * 8 DVE multiply-by-identity + reduce pairs pull the raw diagonal
    dot out of PSUM -- no extra HBM traffic for the diag term.
Two [128, 2048] fp32 PSUM tiles (4 banks each) double-buffer the
matmul -> exp pipeline; scheduler order is pinned with per-iteration
wait hints so the tile scheduler cannot reorder the chain.

Host marshalling: row-normalize p, q (scaled x16 to dodge fp8
subnormals), cast fp8, transpose into [d_partition, k, col] layouts,
rotate q columns.  Host combine: lse = log(esum), exact diagonal
logit = raw_dot * 5/(|u_b||v_b|) (norms of the quantized vectors are
known host-side), then the three scalar means.  The fp8 quantization
noise lands ~1e-4 relative on the final loss, far inside the 2e-2
gate.
"""

import numpy as np

B = 2048
D = 256
NI = 2            # projected views
NJ = 2            # used predicted views
T = 0.2
HL = B // 2       # 1024 rows per core
MT = HL // 128    # 8 row m-tiles per core
PAIRS = [(0, 0), (0, 1), (1, 0), (1, 1)]
DVE_M = ()        # m-tiles whose exp runs on the vector engine (none: the
                  # DVE/Pool exp offloads measured slower than ACT-only)
HALVES = False    # split iterations 0/1 into q-chunk halves for early start
A16 = 2.0 ** 7 / np.log(2.0)       # Schraudolph scale for bf16 bitcast
GAMMA = 1.00151                    # mean rel bias of the bf16 Schraudolph exp

_CACHE = {}


def _patch_tile_drain():
    """This walrus build only accepts 1 sync-wait on a Drain (CTRL_NO)
    instruction, but TileContext's tail drain accumulates one wait per
    active processor.  Split the waits across multiple drains."""
    import concourse.tile as tile
    from concourse.vector_clock import ScopedClock

    if getattr(tile.TileContext, "_drain_split_patch", False):
        return

    def _drain_and_barrier(self, tick_clock, wait_clock):
        nc = self.nc
        drain_inst = nc.sync.drain()
        wait_clock.add_sem_waits(
            drain_inst.ins, ScopedClock({None: tick_clock.global_clock})
        )
        si = drain_inst.ins.sync_info
        if si is not None and si.on_wait and len(si.on_wait) > 1:
            waits = list(si.on_wait)
            si.on_wait = waits[:1]
            for w in waits[1:]:
                extra = nc.sync.drain()
                esi = extra.ins.sync_info
                if esi is None:
                    import concourse.mybir as mybir
                    extra.ins.sync_info = mybir.SyncInfo(on_wait=[w], on_update=[])
                else:
                    esi.on_wait = [w]

        nc.all_engine_barrier()
        assert self.sems is not None
        popped = nc._tile_sem_poison_stack.pop()
        assert popped is self._sem_poison
        nc.clear_and_free_semaphores(list(self.sems.allocated().values()))
        nc.all_engine_barrier()

    tile.TileContext._drain_and_barrier = _drain_and_barrier
    tile.TileContext._drain_split_patch = True


def _split_multiwait(nc, mybir):
    """This walrus build rejects instructions carrying more than one
    semaphore wait.  Hoist excess waits onto standalone EventSemaphore
    instructions inserted just before the original (same engine, in-order
    execution => semantics preserved)."""
    import orjson

    js = orjson.loads(mybir.module_to_json_bytes(nc.m))

    # Delete the Bass-init const-AP memsets and the init all-engine
    # barrier when present (dead weight at startup).
    bb0 = js["functions"][0]["blocks"][0]
    insts = bb0["instructions"]
    ms_idx = [n for n, i in enumerate(insts)
              if i["opcode"] == "Memset"
              and str(i.get("outs", [{}])[0]).find("const-") >= 0]
    if ms_idx:
        lo, hi = ms_idx[0], ms_idx[-1] + 1
        while hi < len(insts) and insts[hi]["opcode"] in ("Drain",
                                                          "EventSemaphore"):
            hi += 1
        bb0["instructions"] = insts[:lo] + insts[hi:]

    ctr = 0
    for f in js["functions"]:
        for bb in f["blocks"]:
            new_insts = []
            for inst in bb["instructions"]:
                si = inst.get("sync_info")
                if si and si.get("on_wait") and len(si["on_wait"]) > 1:
                    waits = si["on_wait"]
                    for w in waits[:-1]:
                        ctr += 1
                        ev = {
                            "engine": inst["engine"],
                            "ins": [],
                            "name": f"WSPLIT-{ctr}",
                            "opcode": "EventSemaphore",
                            "outs": [],
                            "sync_info": {"on_update": [], "on_wait": [w]},
                        }
                        if "debug" in inst:
                            ev["debug"] = inst["debug"]
                        new_insts.append(ev)
                    si["on_wait"] = waits[-1:]
                new_insts.append(inst)
            bb["instructions"] = new_insts
    nc.m = mybir.module_from_json_bytes(orjson.dumps(js))
    return ctr


def _build_program():
    import concourse.bass as bass
    import concourse.tile as tile
    from concourse import mybir
    from contextlib import ExitStack

    _patch_tile_drain()

    fp32 = mybir.dt.float32
    bf16 = mybir.dt.bfloat16
    fp8 = mybir.dt.float8e4
    i16 = mybir.dt.int16
    Exp = mybir.ActivationFunctionType.Exp
    mult = mybir.AluOpType.mult
    add = mybir.AluOpType.add
    X = mybir.AxisListType.X
    DR = mybir.MatmulPerfMode.DoubleRow

    nc = bass.Bass()

    # inputs (host-marshalled: normalized, x16-scaled, fp8, transposed);
    # one fp8 blob [pT8 | qT8] + one fp32 blob [zeros | scl | ident] so
    # startup pays only two DGE setups (one per issuing engine).
    pT_in = nc.dram_tensor("pT8", [128, 2048], fp8, kind="ExternalInput")
    qT_in = nc.dram_tensor("qT8", [128, 4096], fp8, kind="ExternalInput")
    aux_in = nc.dram_tensor("aux", [128, 145], fp32, kind="ExternalInput")
    # combined output: [esums(8) | draws(8) | iter-0/1 half esums(4)]
    outs_t = nc.dram_tensor("outs", [128, 2 * MT + 4], fp32, kind="ExternalOutput")

    with tile.TileContext(nc) as tc, ExitStack() as ctx:
        res = ctx.enter_context(tc.tile_pool(name="res", bufs=1))
        scrap = ctx.enter_context(tc.tile_pool(name="scrap", bufs=2))
        psum = ctx.enter_context(tc.tile_pool(name="psum", bufs=2, space="PSUM"))

        pT8 = res.tile([128, 2, 1024], fp8, tag="pT8")          # [dp, k, b]
        qT8 = res.tile([128, 2, 2, 1024], fp8, tag="qT8")       # [dp, ch, k, c]
        aux = res.tile([128, 145], fp32, tag="aux")
        tmp = res.tile([128, 1], fp32, tag="tmp")
        stats = res.tile([128, 2 * MT + 4], fp32, tag="stats")
        pT = pT8[:]
        qT = qT8[:]
        zb = aux[:, 0:1]
        scl = aux[:, 1:1 + MT]
        sclA = aux[:, 1 + MT:1 + 2 * MT]
        idt = aux[:, 1 + 2 * MT:1 + 2 * MT + 128]
        esums = stats[:, 0:MT]
        draws = stats[:, MT:2 * MT]
        e0h = stats[:, 2 * MT:2 * MT + 4]

        # prefetch the Exp ACT table immediately (tmp is garbage; its
        # output is never consumed -- only the table-load side effect
        # matters, and it runs while the DMAs stream in)
        tc.tile_set_cur_wait(0.0)
        nc.scalar.activation(out=tmp[:], in_=tmp[:], func=Exp, bias=tmp[:])

        # ---- loads (flat APs: one contiguous descriptor per partition) ----
        q_flat = qT8[:].rearrange("p ch k c -> p (ch k c)")
        nc.scalar.dma_start(out=aux[:], in_=aux_in[:])
        nc.sync.dma_start(out=pT8[:].rearrange("p k b -> p (k b)"), in_=pT_in[:])
        nc.sync.dma_start(out=q_flat[:, 0:2048], in_=qT_in[:, 0:2048])
        nc.sync.dma_start(out=q_flat[:, 2048:4096], in_=qT_in[:, 2048:4096])

        # ---- main loop: one (row-tile m) iteration = 4 matmuls ->
        #      exp+rowsum, diag extract on DVE.  Tiles in DVE_M compute
        #      exp via the Schraudolph int16/bf16 bitcast trick: DVE does
        #      the scale+round to int16, GpSimd reduces the bitcast bf16
        #      row -- both off the ACT critical chain.  Scheduler order
        #      is pinned with per-iteration wait hints. ----
        # Iterations 0 and 1 are split into their two 1024-col q chunks,
        # each a full pipeline slot with its own PSUM tile: the ACT chain
        # starts as soon as chunk 0 lands, and the matmul groups behind
        # the short half-exps stay small enough to avoid a transition
        # bubble when full 2048-col iterations begin.
        if HALVES:
            iters = [(0, 0), (0, 1), (1, 0), (1, 1)] + \
                    [(m, None) for m in range(2, MT)]
        else:
            iters = [(m, None) for m in range(MT)]
        for it, (m, half) in enumerate(iters):
            tc.tile_set_cur_wait(0.0004 * (it + 1))
            P = psum.tile([128, 2048], fp32, tag="P", name=f"P{it}")
            if half is None:
                for cc in range(4):
                    ch, ccw = divmod(cc, 2)
                    nc.tensor.matmul(
                        P[:, cc * 512:(cc + 1) * 512],
                        lhsT=pT[:, :, m * 128:(m + 1) * 128],
                        rhs=qT[:, ch, :, ccw * 512:(ccw + 1) * 512],
                        start=True, stop=True,
                        perf_mode=DR,
                    )
                eo = scrap.tile([128, 2048], bf16, tag="eo", name=f"eo{m}")
                nc.scalar.activation(
                    out=eo[:], in_=P[:], func=Exp,
                    scale=scl[:, m:m + 1], bias=zb[:],
                    accum_out=esums[:, m:m + 1],
                )
            else:
                for ccw in range(2):
                    nc.tensor.matmul(
                        P[:, ccw * 512:(ccw + 1) * 512],
                        lhsT=pT[:, :, m * 128:(m + 1) * 128],
                        rhs=qT[:, half, :, ccw * 512:(ccw + 1) * 512],
                        start=True, stop=True,
                        perf_mode=DR,
                    )
                eh = scrap.tile([128, 1024], bf16, tag="eh", name=f"eh{m}{half}")
                nc.scalar.activation(
                    out=eh[:], in_=P[:, 0:1024],
                    func=Exp, scale=scl[:, m:m + 1], bias=zb[:],
                    accum_out=e0h[:, m * 2 + half:m * 2 + half + 1],
                )
            if half != 1:
                dg = scrap.tile([128, 128], fp32, tag="dg", name=f"dg{m}")
                nc.vector.tensor_mul(dg[:], P[:, m * 128:(m + 1) * 128], idt[:])
                nc.vector.tensor_reduce(
                    out=draws[:, m:m + 1], in_=dg[:], axis=X, op=add)

        nc.sync.dma_start(out=outs_t[:], in_=stats[:])

    _split_multiwait(nc, mybir)
    return nc


def _get_program():
    if "nc" not in _CACHE:
        _CACHE["nc"] = _build_program()
    return _CACHE["nc"]


def _marshal(projected, predicted):
    import ml_dtypes

    f8 = ml_dtypes.float8_e4m3

    p = np.ascontiguousarray(projected, dtype=np.float32)          # [B, 2, 256]
    q = np.ascontiguousarray(predicted[:, :NJ, :], dtype=np.float32)

    pn = 16.0 * p / np.linalg.norm(p, axis=-1, keepdims=True)
    qn = 16.0 * q / np.linalg.norm(q, axis=-1, keepdims=True)
    u8 = pn.astype(f8)                                             # [B, 2, 256]
    v8 = qn.astype(f8)
    u = u8.astype(np.float32)
    v = v8.astype(np.float32)
    unorm = np.linalg.norm(u, axis=-1)                             # [B, 2]
    vnorm = np.linalg.norm(v, axis=-1)                             # [B, 2]

    eye = np.eye(128, dtype=np.float32)
    zeros = np.zeros((128, 1), dtype=np.float32)

    in_maps = []
    dscale = []        # host-side diag logit scale per core: [128, MT]
    for (i, j) in PAIRS:
        for h in range(2):
            rows = slice(h * HL, (h + 1) * HL)
            # pT8 [dp, k, b]: d = k*128 + dp
            A = u8[rows, i, :].reshape(HL, 2, 128)                 # [b, k, dp]
            pT8 = np.ascontiguousarray(A.transpose(2, 1, 0)).reshape(128, 2 * HL)
            # qT8 [dp, ch, k, c] with columns rotated by h*HL
            cols = (np.arange(B) + h * HL) % B
            Bm = v8[cols, j, :].reshape(2, 1024, 2, 128)           # [ch, c, k, dp]
            qT8 = np.ascontiguousarray(Bm.transpose(3, 0, 2, 1)).reshape(128, 4096)
            # per-row exp scale 5/(16*|u_b|), laid out [dp=row%128, m]
            un = unorm[rows, i].reshape(MT, 128)                   # [m, pp]
            scl = np.ascontiguousarray((5.0 / (16.0 * un)).T)      # [128, MT]
            # host diag scale 5/(|u_b| |v_b|)
            vn = vnorm[rows, j].reshape(MT, 128)
            dscale.append(5.0 / (un * vn).T)                       # [128, MT]
            aux = np.concatenate(
                [zeros, scl.astype(np.float32),
                 (scl * A16).astype(np.float32), eye], axis=1)
            in_maps.append({"pT8": pT8, "qT8": qT8,
                            "aux": np.ascontiguousarray(aux)})
    return in_maps, dscale


def kernel(projected, predicted, _trace=False):
    from concourse.bass_utils import run_bass_kernel_spmd

    nc = _get_program()
    in_maps, dscale = _marshal(projected, predicted)
    out = run_bass_kernel_spmd(nc, in_maps, list(range(8)), trace=_trace)
    results = out.results
    if _trace:
        _CACHE["last_bkr"] = out

    # ---- host combine ----
    Lsum = np.zeros((NI, NJ), dtype=np.float64)
    for pi, (i, j) in enumerate(PAIRS):
        for h in range(2):
            r = results[pi * 2 + h]["outs"].astype(np.float64)     # [128, 20]
            esum = r[:, 0:MT].copy()
            if HALVES:
                esum[:, 0] = r[:, 2 * MT] + r[:, 2 * MT + 1]
                esum[:, 1] = r[:, 2 * MT + 2] + r[:, 2 * MT + 3]
            draw = r[:, MT:2 * MT]
            lse = np.log(esum)
            dlog = draw * dscale[pi * 2 + h]
            Lsum[i, j] += np.sum(lse - dlog)
    L = Lsum / B

    global_sum = L[0, 1] + L[1, 0]
    local_sum = L[0, 0] + L[0, 1] + L[1, 0] + L[1, 1]
    global_loss = global_sum / 2.0
    local_loss = local_sum / 4.0
    total = (global_sum + local_sum) / 6.0
    return np.array([total, global_loss, local_loss], dtype=np.float32)


# revision 37
# speedup vs baseline: 1.1192x; 1.1192x over previous
"""Exact V1 variant (first fp8 DoubleRow version, measured 33392ns).
Separate inputs, strided chunk DMAs, no scheduler pins, table prefetch
gated on the zeros DMA."""

import numpy as np

B = 2048
D = 256
NI = 2
NJ = 2
T = 0.2
HL = B // 2
MT = HL // 128
PAIRS = [(0, 0), (0, 1), (1, 0), (1, 1)]

_CACHE = {}

from kernel_shipped import _patch_tile_drain, _split_multiwait


def _build_program():
    import concourse.bass as bass
    import concourse.tile as tile
    from concourse import mybir
    from contextlib import ExitStack

    _patch_tile_drain()

    fp32 = mybir.dt.float32
    bf16 = mybir.dt.bfloat16
    fp8 = mybir.dt.float8e4
    Exp = mybir.ActivationFunctionType.Exp
    mult = mybir.AluOpType.mult
    add = mybir.AluOpType.add
    X = mybir.AxisListType.X
    DR = mybir.MatmulPerfMode.DoubleRow

    nc = bass.Bass()

    pT_in = nc.dram_tensor("pT8", [128, 2 * HL], fp8, kind="ExternalInput")
    qT_in = nc.dram_tensor("qT8", [128, 2 * 2 * 1024], fp8, kind="ExternalInput")
    id_in = nc.dram_tensor("ident", [128, 128], fp32, kind="ExternalInput")
    sc_in = nc.dram_tensor("scl", [128, MT], fp32, kind="ExternalInput")
    zr_in = nc.dram_tensor("zeros", [128, 1], fp32, kind="ExternalInput")
    outs_t = nc.dram_tensor("outs", [128, 2 * MT], fp32, kind="ExternalOutput")

    with tile.TileContext(nc) as tc, ExitStack() as ctx:
        res = ctx.enter_context(tc.tile_pool(name="res", bufs=1))
        scrap = ctx.enter_context(tc.tile_pool(name="scrap", bufs=2))
        psum = ctx.enter_context(tc.tile_pool(name="psum", bufs=2, space="PSUM"))

        pT8 = res.tile([128, 2, HL], fp8, tag="pT")
        qT8 = res.tile([128, 2, 2, 1024], fp8, tag="qT")
        idt = res.tile([128, 128], fp32, tag="idt")
        scl = res.tile([128, MT], fp32, tag="scl")
        zb = res.tile([128, 1], fp32, tag="zb")
        tmp = res.tile([128, 1], fp32, tag="tmp")
        stats = res.tile([128, 2 * MT], fp32, tag="stats")
        esums = stats[:, 0:MT]
        draws = stats[:, MT:2 * MT]

        nc.scalar.dma_start(out=zb[:], in_=zr_in[:])
        nc.scalar.dma_start(out=scl[:], in_=sc_in[:])
        nc.scalar.dma_start(out=idt[:], in_=id_in[:])
        nc.sync.dma_start(out=pT8[:], in_=pT_in.rearrange("p (k b) -> p k b", k=2))
        q_src = qT_in.rearrange("p (ch k c) -> p ch k c", ch=2, k=2)
        nc.sync.dma_start(out=qT8[:, 0], in_=q_src[:, 0])
        nc.sync.dma_start(out=qT8[:, 1], in_=q_src[:, 1])

        nc.scalar.activation(out=tmp[:], in_=zb[:], func=Exp, bias=zb[:])

        for m in range(MT):
            P = psum.tile([128, 2048], fp32, tag="P", name=f"P{m}")
            for cc in range(4):
                ch, ccw = divmod(cc, 2)
                nc.tensor.matmul(
                    P[:, cc * 512:(cc + 1) * 512],
                    lhsT=pT8[:, :, m * 128:(m + 1) * 128],
                    rhs=qT8[:, ch, :, ccw * 512:(ccw + 1) * 512],
                    start=True, stop=True,
                    perf_mode=DR,
                )
            eo = scrap.tile([128, 2048], bf16, tag="eo", name=f"eo{m}")
            nc.scalar.activation(
                out=eo[:], in_=P[:], func=Exp,
                scale=scl[:, m:m + 1], bias=zb[:],
                accum_out=esums[:, m:m + 1],
            )
            dg = scrap.tile([128, 128], fp32, tag="dg", name=f"dg{m}")
            nc.vector.tensor_mul(dg[:], P[:, m * 128:(m + 1) * 128], idt[:])
            nc.vector.tensor_reduce(
                out=draws[:, m:m + 1], in_=dg[:], axis=X, op=add)

        nc.sync.dma_start(out=outs_t[:], in_=stats[:])

    _split_multiwait(nc, mybir)
    return nc


def _get_program():
    if "nc" not in _CACHE:
        _CACHE["nc"] = _build_program()
    return _CACHE["nc"]


def _marshal(projected, predicted):
    import ml_dtypes

    f8 = ml_dtypes.float8_e4m3
    p = np.ascontiguousarray(projected, dtype=np.float32)
    q = np.ascontiguousarray(predicted[:, :NJ, :], dtype=np.float32)
    pn = 16.0 * p / np.linalg.norm(p, axis=-1, keepdims=True)
    qn = 16.0 * q / np.linalg.norm(q, axis=-1, keepdims=True)
    u8 = pn.astype(f8)
    v8 = qn.astype(f8)
    u = u8.astype(np.float32)
    v = v8.astype(np.float32)
    unorm = np.linalg.norm(u, axis=-1)
    vnorm = np.linalg.norm(v, axis=-1)

    eye = np.eye(128, dtype=np.float32)
    zeros = np.zeros((128, 1), dtype=np.float32)

    in_maps = []
    dscale = []
    for (i, j) in PAIRS:
        for h in range(2):
            rows = slice(h * HL, (h + 1) * HL)
            A = u8[rows, i, :].reshape(HL, 2, 128)
            pT8 = np.ascontiguousarray(A.transpose(2, 1, 0)).reshape(128, 2 * HL)
            cols = (np.arange(B) + h * HL) % B
            Bm = v8[cols, j, :].reshape(2, 1024, 2, 128)
            qT8 = np.ascontiguousarray(Bm.transpose(3, 0, 2, 1)).reshape(128, 4096)
            un = unorm[rows, i].reshape(MT, 128)
            scl = np.ascontiguousarray((5.0 / (16.0 * un)).T)
            vn = vnorm[rows, j].reshape(MT, 128)
            dscale.append(5.0 / (un * vn).T)
            in_maps.append({
                "pT8": pT8,
                "qT8": qT8,
                "ident": eye,
                "scl": scl.astype(np.float32),
                "zeros": zeros,
            })
    return in_maps, dscale


def kernel(projected, predicted, _trace=False):
    from concourse.bass_utils import run_bass_kernel_spmd

    nc = _get_program()
    in_maps, dscale = _marshal(projected, predicted)
    out = run_bass_kernel_spmd(nc, in_maps, list(range(8)), trace=_trace)
    results = out.results
    if _trace:
        _CACHE["last_bkr"] = out

    Lsum = np.zeros((NI, NJ), dtype=np.float64)
    for pi, (i, j) in enumerate(PAIRS):
        for h in range(2):
            r = results[pi * 2 + h]["outs"].astype(np.float64)
            esum = r[:, 0:MT]
            draw = r[:, MT:2 * MT]
            lse = np.log(esum)
            dlog = draw * dscale[pi * 2 + h]
            Lsum[i, j] += np.sum(lse - dlog)
    L = Lsum / B

    global_sum = L[0, 1] + L[1, 0]
    local_sum = L[0, 0] + L[0, 1] + L[1, 0] + L[1, 1]
    return np.array([(global_sum + local_sum) / 6.0,
                     global_sum / 2.0, local_sum / 4.0], dtype=np.float32)
